# revision 3
# baseline (speedup 1.0000x reference)
"""CRF head kernel for Trainium2 (Bass/Tile), 8-core data-parallel.

Computes: out[b, t, :] = x[b, t, :] + transitions[argmax(x[b, t, :]), :]
for x of shape [128, 1024, 256] f32 and transitions [256, 256] f32.

Sharding: batch dim split across 8 NeuronCores (16 batches / core).
Per core: 16*1024 = 16384 rows, processed in megatiles of 1024 rows laid
out as [128 partitions, 8 groups, 256 tags] (each partition holds 8
consecutive rows -> contiguous 8KB DMA descriptors per partition).

Strategy: instead of a row-gather from HBM (16.8MB of SWDGE traffic),
select transitions rows with a one-hot matmul on the TensorEngine:
  out = x + onehot(argmax(x)) @ transitions            (bf16 matmul, f32 PSUM)

Per megatile:
  1. HWDGE load 1MB x tile (sync ring).
  2. GpSimd: reduce_max over tags -> mx [128, 8].
  3. DVE: broadcast mx to [128, 8, 8], then 8 per-group max_index calls
     (first-occurrence argmax within each 256-wide group — exactly
     jnp.argmax semantics, no cross-group value-collision hazard).
  4. Cast indices to u8, round-trip through DRAM to broadcast them to all
     128 partitions (scalar/ACT HWDGE ring, FIFO-ordered): idxb[k, r].
  5. GpSimd: onehotT[k, r] = (idxb[k, r] == k) for k in [0,128) and
     [128,256) -> two [128, 1024] bf16 one-hot-transposed tiles.
  6. TensorE: per group g, G[p, :] = onehotT0[:, rows_g].T @ transA
     + onehotT1[:, rows_g].T @ transB accumulated in PSUM (f32).
  7. DVE: x += G per group (reads PSUM), HWDGE store 1MB back.
"""

import sys

for _p in ("/opt/trn_rl_repo",):
    if _p not in sys.path:
        sys.path.append(_p)

import numpy as np

import concourse.bass as bass
import concourse.bacc as bacc
import concourse.mybir as mybir
import concourse.tile as tile
import concourse.bass_utils as bass_utils

N_CORES = 8
B, T, TAGS = 128, 1024, 256
R = (B // N_CORES) * T          # rows per core = 16384
P = 128                         # SBUF partitions
G = 8                           # rows per partition per megatile
ROWS_PER_MT = P * G             # 1024
M = R // ROWS_PER_MT            # 16 megatiles per core

_CACHE = {}


def _build():
    nc = bacc.Bacc("TRN2", target_bir_lowering=False, debug=False)

    x = nc.dram_tensor("x", [R, TAGS], mybir.dt.float32, kind="ExternalInput")
    t = nc.dram_tensor("t", [TAGS, TAGS], mybir.dt.float32, kind="ExternalInput")
    y = nc.dram_tensor("y", [R, TAGS], mybir.dt.float32, kind="ExternalOutput")

    # megatile m, partition p holds rows m*1024 + p*G .. +G-1 (contiguous)
    xv = x.ap().rearrange("(m p g) d -> m p (g d)", p=P, g=G)
    yv = y.ap().rearrange("(m p g) d -> m p (g d)", p=P, g=G)
    tv = t.ap().rearrange("(h k) d -> h k d", h=2)   # two 128-row chunks

    with tile.TileContext(nc) as tc:
        with (
            tc.tile_pool(name="cp", bufs=1) as cp,
            tc.tile_pool(name="xp", bufs=4) as xp,
            tc.tile_pool(name="sp", bufs=4) as sp,
            tc.tile_pool(name="op", bufs=2) as op,
            tc.tile_pool(name="pp", bufs=1, space="PSUM") as pp,
            tc.tile_pool(name="dp", bufs=4, space="DRAM") as dp,
        ):
            # transitions, f32 -> bf16, split into two [128, 256] k-chunks
            t_f32 = cp.tile([P, 2 * TAGS], mybir.dt.float32, tag="tf", name="t_f32")
            nc.sync.dma_start(out=t_f32[:].rearrange("k (h d) -> k h d", h=2),
                              in_=tv.rearrange("h k d -> k h d"))
            t_bf = cp.tile([P, 2 * TAGS], mybir.dt.bfloat16, tag="tb", name="t_bf")
            nc.vector.tensor_copy(out=t_bf[:], in_=t_f32[:])
            tA = t_bf[:, 0:TAGS]          # transitions[0:128, :]   (k on partitions)
            tB = t_bf[:, TAGS:2 * TAGS]   # transitions[128:256, :]

            # per-partition iota as f32 scalar operands: k and k+128
            ki = cp.tile([P, 2], mybir.dt.int32, tag="ki", name="ki")
            nc.gpsimd.iota(ki[:], pattern=[[128, 2]], base=0, channel_multiplier=1)
            kf = cp.tile([P, 2], mybir.dt.float32, tag="kf", name="kf")
            nc.vector.tensor_copy(out=kf[:], in_=ki[:])

            for m in range(M):
                x_t = xp.tile([P, G * TAGS], mybir.dt.float32, tag="x",
                              name=f"x_{m}")
                nc.sync.dma_start(out=x_t[:], in_=xv[m])

                x3 = x_t[:].rearrange("p (c d) -> p c d", d=TAGS)
                mx = sp.tile([P, G], mybir.dt.float32, tag="mx", name=f"mx_{m}")
                nc.vector.tensor_reduce(out=mx[:], in_=x3,
                                        axis=mybir.AxisListType.X,
                                        op=mybir.AluOpType.max)

                # mx8[p, c, i] = mx[p, c] (each group max replicated 8x for
                # max_index's fixed 8-slot in_max operand)
                mx8 = sp.tile([P, G * 8], mybir.dt.float32, tag="mx8",
                              name=f"mx8_{m}")
                mx_ap = mx[:]
                mx_b = bass.AP(mx_ap.tensor, mx_ap.offset,
                               [mx_ap.ap[0], [1, G], [0, 8]])
                nc.vector.tensor_copy(
                    out=mx8[:].rearrange("p (c i) -> p c i", i=8), in_=mx_b)

                # per-group first-occurrence argmax (matches jnp.argmax)
                scr = sp.tile([P, G * 8], mybir.dt.uint16, tag="scr",
                              name=f"scr_{m}")
                for c in range(G):
                    nc.vector.max_index(out=scr[:, c * 8:(c + 1) * 8],
                                        in_max=mx8[:, c * 8:(c + 1) * 8],
                                        in_values=x3[:, c, :])
                idx8 = sp.tile([P, G], mybir.dt.uint8, tag="idx8",
                               name=f"idx8_{m}")
                nc.vector.tensor_copy(
                    out=idx8[:],
                    in_=scr[:].rearrange("p (c i) -> p c i", i=8)[:, :, 0])

                # broadcast indices to all partitions via DRAM round-trip
                # (both DMAs on the ACT HWDGE ring -> FIFO ordering)
                img = dp.tile([ROWS_PER_MT], mybir.dt.uint8, tag="img",
                              name=f"img_{m}")
                nc.scalar.dma_start(
                    out=img[:].rearrange("(p c) -> p c", p=P), in_=idx8[:])
                idxb = sp.tile([P, ROWS_PER_MT], mybir.dt.uint8, tag="idxb",
                               name=f"idxb_{m}")
                img_rep = bass.AP(img[:].tensor, img[:].offset,
                                  [[0, P], [1, ROWS_PER_MT]])
                nc.scalar.dma_start(out=idxb[:], in_=img_rep)

                # transposed one-hot: oh0[k, r] = (idx[r] == k), k in [0,128)
                #                     oh1[k, r] = (idx[r] == k+128)
                oh0 = op.tile([P, ROWS_PER_MT], mybir.dt.bfloat16, tag="oh0",
                              name=f"oh0_{m}")
                oh1 = op.tile([P, ROWS_PER_MT], mybir.dt.bfloat16, tag="oh1",
                              name=f"oh1_{m}")
                nc.gpsimd.tensor_scalar(out=oh0[:], in0=idxb[:],
                                        scalar1=kf[:, 0:1], scalar2=None,
                                        op0=mybir.AluOpType.is_equal)
                nc.gpsimd.tensor_scalar(out=oh1[:], in0=idxb[:],
                                        scalar1=kf[:, 1:2], scalar2=None,
                                        op0=mybir.AluOpType.is_equal)

                # row r = p*8 + c  ->  group c rows are stride-8 slices
                oh0v = oh0[:].rearrange("k (p c) -> k c p", c=G)
                oh1v = oh1[:].rearrange("k (p c) -> k c p", c=G)

                for c in range(G):
                    ps = pp.tile([P, 512], mybir.dt.float32, tag=f"ps{c}",
                                 name=f"ps{c}_{m}")
                    nc.tensor.matmul(out=ps[:, 0:TAGS], lhsT=oh0v[:, c, :],
                                     rhs=tA, start=True, stop=False)
                    nc.tensor.matmul(out=ps[:, 0:TAGS], lhsT=oh1v[:, c, :],
                                     rhs=tB, start=False, stop=True)
                    nc.vector.tensor_tensor(
                        out=x_t[:, c * TAGS:(c + 1) * TAGS],
                        in0=x_t[:, c * TAGS:(c + 1) * TAGS],
                        in1=ps[:, 0:TAGS],
                        op=mybir.AluOpType.add)

                nc.sync.dma_start(out=yv[m], in_=x_t[:])

    nc.compile()
    return nc


def get_nc():
    if "nc" not in _CACHE:
        _CACHE["nc"] = _build()
    return _CACHE["nc"]


def kernel(launch_matrix, transitions):
    launch = np.ascontiguousarray(np.asarray(launch_matrix, dtype=np.float32))
    trans = np.ascontiguousarray(np.asarray(transitions, dtype=np.float32))
    assert launch.shape == (B, T, TAGS), launch.shape
    assert trans.shape == (TAGS, TAGS), trans.shape

    nc = get_nc()
    shards = launch.reshape(N_CORES, R, TAGS)
    in_maps = [{"x": shards[c], "t": trans} for c in range(N_CORES)]
    res = bass_utils.run_bass_kernel_spmd(nc, in_maps,
                                          core_ids=list(range(N_CORES)))
    _CACHE["last_results"] = res
    out = np.concatenate([res.results[c]["y"] for c in range(N_CORES)], axis=0)
    return out.reshape(B, T, TAGS)
